# revision 12
# baseline (speedup 1.0000x reference)
"""BatchWhiten Trainium2 kernel (8-core SPMD, Bass/Tile).

y = x @ inv_sqrtm(max(0.1*running_covar + 0.9*(x^T x)/N, 1e-5))

Strategy (data-parallel over rows):
  - shard x row-wise across 8 cores
  - phase 1 (per core): stream x f32 via HWDGE, cast f32->fp16 on DVE,
    accumulate S_c = x_c^T x_c upper-triangular blocks in fp32 PSUM
    (symmetric; lower blocks reconstructed after the reduce), and
    PE-transpose every x tile, spilling compact fp16 xT tiles to a DRAM
    scratch (halves phase-2 input bytes). The last DEFER chunks'
    transposes are emitted after the AllReduce call so the PE has work
    during the collective.
  - AllReduce the 512x512 partial (pre-scaled by 0.9/N)
  - replicated inverse-sqrt via coupled Newton-Schulz (fp16) + one fp32
    polish step (no SVD: spectrum is ~[0.95, 1.4]); phase-2 xT chunks
    prefetch during this section (pure DMA, no compute needed)
  - phase 2: y_tile = xT_tile^T @ B via fp16 matmuls, fp32 out

PSUM budget (8 banks): phase1 covps 4 + xt_ps 2 -> nsps 3 + y_ps 3.
"""

import numpy as np

import concourse.bacc as bacc
import concourse.tile as tile
import concourse.mybir as mybir
from concourse import bass_utils

N_CORES = 8
D = 512
P = 128
MC = D // P              # 4 feature chunks of 128
N_TOTAL = 262144
SHARD = N_TOTAL // N_CORES
G = 8                    # row-tiles (128 rows each) per DMA chunk
DEFER = 7                # trailing chunks whose transposes overlap the AR
TAIL = 6                 # trailing chunks reduced separately (fp16, packed)
MOMENTUM = 0.1
EPS = 1e-5
NS_ITERS = 5
POLISH = False
C_SCALE = 2.0            # spectral normalizer for Newton-Schulz
INV_SQRT_C = 1.0 / np.sqrt(C_SCALE)

f32 = mybir.dt.float32
f16 = mybir.dt.float16


def _cols(mi):
    return slice(mi * P, (mi + 1) * P)


def build_program(shard=SHARD, n_total=N_TOTAL, ns_iters=NS_ITERS, g=G,
                  defer=DEFER, polish=POLISH, tail=TAIL):
    """Build the SPMD Bass program. Returns compiled Bacc instance."""
    tpc = shard // P          # row-tiles per core
    nchunk = tpc // g
    assert nchunk * g == tpc
    defer = min(defer, nchunk)
    tail = min(tail, max(nchunk - 1, 0)) or 1

    nc = bacc.Bacc(
        "TRN2", target_bir_lowering=False, debug=False, num_devices=N_CORES
    )
    x_d = nc.dram_tensor("x", [shard, D], f32, kind="ExternalInput")
    rc_d = nc.dram_tensor("running_covar", [D, D], f32, kind="ExternalInput")
    eye15_d = nc.dram_tensor("eye15", [D, D], f16, kind="ExternalInput")
    id16_d = nc.dram_tensor("id128_16", [P, P], f16, kind="ExternalInput")
    id32_d = nc.dram_tensor("id128_32", [P, P], f32, kind="ExternalInput")
    eye16_d = nc.dram_tensor("eye16", [D, D], f16, kind="ExternalInput")
    y_d = nc.dram_tensor("y", [shard, D], f32, kind="ExternalOutput")

    # partition-major DRAM views: [p, tile_idx, feat]
    x_v = x_d.ap().rearrange("(n p) m -> p n m", p=P)
    y_v = y_d.ap().rearrange("(n p) m -> p n m", p=P)
    rc_v = rc_d.ap().rearrange("(t p) m -> p t m", p=P)
    e15_v = eye15_d.ap().rearrange("(t p) m -> p t m", p=P)
    e16_v = eye16_d.ap().rearrange("(t p) m -> p t m", p=P)

    cov_scale = (1.0 - MOMENTUM) / float(n_total)

    with tile.TileContext(nc) as tc:
        with (
            tc.tile_pool(name="const", bufs=1) as constp,
            tc.tile_pool(name="dram", bufs=1, space="DRAM") as dramp,
        ):
            # ---- constants ----
            id16 = constp.tile([P, P], f16, name="id16")
            nc.sync.dma_start(id16[:], id16_d.ap())
            id32 = constp.tile([P, P], f32, name="id32")
            nc.sync.dma_start(id32[:], id32_d.ap())
            e15 = []
            for mi in range(MC):
                t = constp.tile([P, D], f16, name=f"e15_{mi}")
                nc.sync.dma_start(t[:], e15_v[:, mi, :])
                e15.append(t)
            B16 = [
                constp.tile([P, D], f16, name=f"b16_{mi}") for mi in range(MC)
            ]
            # fp16 transposed-x scratch, one DRAM tile per chunk
            xt_scr = [
                dramp.tile([g * P, D], f16, name=f"xts_{c}")
                for c in range(nchunk)
            ]
            xt_scr_v = [t.rearrange("(n p) m -> p n m", p=P) for t in xt_scr]

            # ---- phase 1 ----
            with (
                tc.tile_pool(name="covps", bufs=1, space="PSUM") as covps,
                tc.tile_pool(name="xtps", bufs=2, space="PSUM") as xtpsp,
                tc.tile_pool(name="p1xf", bufs=6) as p1xfp,
                tc.tile_pool(name="p1x16", bufs=defer + 3) as p1x16p,
                tc.tile_pool(name="p1xts", bufs=3) as p1xtsp,
                tc.tile_pool(name="mid", bufs=1) as midp,
                nc.named_scope("phase1"),
            ):
                cov_ps = [
                    covps.tile([P, D], f32, name=f"cov{mi}") for mi in range(MC)
                ]

                def transpose_chunk(c, x16):
                    xts = p1xtsp.tile([P, g, D], f16, name="xts", tag="xts")
                    for j0 in range(0, g, 2):
                        tps = xtpsp.tile([P, 2, D], f16, name="xt_ps", tag="xt_ps")
                        for jj in range(2):
                            for ki in range(MC):
                                nc.tensor.transpose(
                                    tps[:, jj, _cols(ki)],
                                    x16[:, j0 + jj, _cols(ki)],
                                    id16[:],
                                )
                        nc.vector.tensor_copy(xts[:, j0 : j0 + 2, :], tps[:])
                    nc.sync.dma_start(xt_scr_v[c][:, :, :], xts[:])

                deferred = []
                hc = g // 2
                n_a = nchunk - tail          # chunks in the main accumulator
                for c in range(nchunk):
                    x16 = p1x16p.tile([P, g, D], f16, name="p1x16", tag="p1x16")
                    for h in range(2):
                        xf = p1xfp.tile([P, hc, D], f32, name="p1xf", tag="p1xf")
                        nc.scalar.dma_start(
                            xf[:],
                            x_v[:, c * g + h * hc : c * g + (h + 1) * hc, :],
                        )
                        nc.vector.tensor_copy(
                            x16[:, h * hc : (h + 1) * hc, :], xf[:]
                        )
                    for j in range(g):
                        t = c * g + j
                        for mi in range(MC):
                            nc.tensor.matmul(
                                cov_ps[mi][:, mi * P :],
                                x16[:, j, _cols(mi)],
                                x16[:, j, mi * P :],
                                start=(t == 0 or t == n_a * g),
                                stop=(t == n_a * g - 1 or t == tpc - 1),
                            )
                    if c < nchunk - defer:
                        transpose_chunk(c, x16)
                    else:
                        deferred.append((c, x16))

                    if c == n_a - 1:
                        # ---- main AllReduce (f32): snapshot of the partial
                        # accumulation after n_a chunks, launched early ----
                        s_stage = midp.tile([P, MC, D], f32, name="s_stage")
                        for mi in range(MC):
                            if mi:
                                nc.vector.memset(s_stage[:, mi, : mi * P], 0.0)
                            nc.vector.tensor_scalar_mul(
                                s_stage[:, mi, mi * P :],
                                cov_ps[mi][:, mi * P :],
                                cov_scale,
                            )
                        cc_in = dramp.tile([D, D], f32, name="cc_in")
                        cc_out = dramp.tile(
                            [D, D], f32, name="cc_out", addr_space="Shared"
                        )
                        cc_in_v = cc_in.rearrange("(t p) m -> p t m", p=P)
                        cc_out_v = cc_out.rearrange("(t p) m -> p t m", p=P)
                        nc.sync.dma_start(cc_in_v[:, :, :], s_stage[:])
                        nc.gpsimd.collective_compute(
                            "AllReduce",
                            mybir.AluOpType.add,
                            replica_groups=[list(range(N_CORES))],
                            ins=[cc_in[:]],
                            outs=[cc_out[:]],
                        )

                # ---- tail AllReduce (fp16, packed upper-tri blocks):
                # tail = full*scale - snapshot ----
                UBLK = [(mi, mj) for mi in range(MC) for mj in range(mi, MC)]
                s_tail = midp.tile([P, len(UBLK) * P], f16, name="s_tail")
                for b, (mi, mj) in enumerate(UBLK):
                    nc.vector.tensor_scalar_mul(
                        s_tail[:, b * P : (b + 1) * P],
                        cov_ps[mi][:, mj * P : (mj + 1) * P],
                        cov_scale,
                    )
                cc_tin = dramp.tile([len(UBLK) * P, P], f16, name="cc_tin")
                cc_tout = dramp.tile(
                    [len(UBLK) * P, P], f16, name="cc_tout", addr_space="Shared"
                )
                cc_tin_v = cc_tin.rearrange("(t p) m -> p t m", p=P)
                cc_tout_v = cc_tout.rearrange("(t p) m -> p t m", p=P)
                nc.sync.dma_start(cc_tin_v[:, :, :], s_tail[:])
                nc.gpsimd.collective_compute(
                    "AllReduce",
                    mybir.AluOpType.add,
                    replica_groups=[list(range(N_CORES))],
                    ins=[cc_tin[:]],
                    outs=[cc_tout[:]],
                )

                # deferred transposes execute on the PE while the ARs run
                for c, x16 in deferred:
                    transpose_chunk(c, x16)

            # ---- phase-2 prefetch pool (loads overlap the NS section) ----
            with tc.tile_pool(name="p2xt", bufs=16) as p2xtp:
                p2_chunks = []
                for c in range(nchunk):
                    xtin = p2xtp.tile([P, g, D], f16, name="xtin", tag="xtin")
                    nc.gpsimd.dma_start(xtin[:], xt_scr_v[c][:, :, :])
                    p2_chunks.append(xtin)

                # ---- NS section ----
                with (
                    tc.tile_pool(name="nsstate", bufs=2) as nsp,
                    tc.tile_pool(name="ns32", bufs=1) as ns32p,
                    tc.tile_pool(name="nstmp", bufs=2) as nstmpp,
                    tc.tile_pool(name="mid2", bufs=1) as mid2p,
                    tc.tile_pool(name="nsps", bufs=3, space="PSUM") as nsps,
                    nc.named_scope("ns"),
                ):
                    s_sum = mid2p.tile([P, MC, D], f32, name="s_sum")
                    nc.sync.dma_start(s_sum[:], cc_out_v[:, :, :])
                    st_sum = mid2p.tile([P, len(UBLK) * P], f16, name="st_sum")
                    nc.sync.dma_start(st_sum[:], cc_tout_v[:, :, :])
                    for b, (mi, mj) in enumerate(UBLK):
                        nc.vector.tensor_tensor(
                            s_sum[:, mi, mj * P : (mj + 1) * P],
                            s_sum[:, mi, mj * P : (mj + 1) * P],
                            st_sum[:, b * P : (b + 1) * P],
                            mybir.AluOpType.add,
                        )
                    # reconstruct lower-tri blocks: S[mj][mi] = S[mi][mj]^T
                    for mi in range(MC):
                        for mj in range(mi + 1, MC):
                            rps = nsps.tile(
                                [P, P], f32, name="rec_ps", tag="ns_ps"
                            )
                            nc.tensor.transpose(
                                rps[:], s_sum[:, mi, _cols(mj)], id32[:]
                            )
                            nc.vector.tensor_copy(s_sum[:, mj, _cols(mi)], rps[:])

                    # C = max(0.9*covar + 0.1*rc, EPS); A = C / C_SCALE
                    rc_sb = mid2p.tile([P, MC, D], f32, name="rc_sb")
                    nc.sync.dma_start(rc_sb[:], rc_v[:, :, :])
                    a32 = []
                    for mi in range(MC):
                        t32 = ns32p.tile([P, D], f32, name=f"a32_{mi}")
                        nc.vector.tensor_scalar_mul(
                            t32[:], rc_sb[:, mi, :], MOMENTUM
                        )
                        nc.vector.tensor_tensor(
                            t32[:], t32[:], s_sum[:, mi, :],
                            mybir.AluOpType.add,
                        )
                        nc.vector.tensor_scalar(
                            t32[:], t32[:], EPS, 1.0 / C_SCALE,
                            mybir.AluOpType.max, mybir.AluOpType.mult,
                        )
                        a32.append(t32)

                    # Newton-Schulz (fp16): Y0 = A, Z0 = I
                    Y, Z = [], []
                    for mi in range(MC):
                        y0 = nsp.tile([P, D], f16, name=f"y0_{mi}", tag=f"Y{mi}")
                        nc.vector.tensor_copy(y0[:], a32[mi][:])
                        Y.append(y0)
                        z0 = nsp.tile([P, D], f16, name=f"z0_{mi}", tag=f"Z{mi}")
                        nc.sync.dma_start(z0[:], e16_v[:, mi, :])
                        Z.append(z0)

                    for it in range(ns_iters):
                        last = it == ns_iters - 1
                        T = []
                        for mi in range(MC):
                            pps = nsps.tile([P, D], f32, name="ns_ps", tag="ns_ps")
                            for ki in range(MC):
                                nc.tensor.matmul(
                                    pps[:],
                                    Z[ki][:, _cols(mi)],
                                    Y[ki][:],
                                    start=(ki == 0),
                                    stop=(ki == MC - 1),
                                )
                            tt = nsp.tile([P, D], f16, name=f"t_{mi}", tag=f"T{mi}")
                            nc.vector.tensor_scalar_mul(tt[:], pps[:], -0.5)
                            nc.vector.tensor_tensor(
                                tt[:], tt[:], e15[mi][:], mybir.AluOpType.add
                            )
                            T.append(tt)
                        newY, newZ = [], []
                        for mi in range(MC):
                            if not last:
                                yps = nsps.tile(
                                    [P, D], f32, name="ns_ps", tag="ns_ps"
                                )
                                for ki in range(MC):
                                    nc.tensor.matmul(
                                        yps[:],
                                        Y[ki][:, _cols(mi)],
                                        T[ki][:],
                                        start=(ki == 0),
                                        stop=(ki == MC - 1),
                                    )
                                ny = nsp.tile(
                                    [P, D], f16, name=f"ny_{mi}", tag=f"Y{mi}"
                                )
                                nc.vector.tensor_copy(ny[:], yps[:])
                                newY.append(ny)

                            zps = nsps.tile([P, D], f32, name="ns_ps", tag="ns_ps")
                            for ki in range(MC):
                                nc.tensor.matmul(
                                    zps[:],
                                    T[ki][:, _cols(mi)],
                                    Z[ki][:],
                                    start=(ki == 0),
                                    stop=(ki == MC - 1),
                                )
                            nz = nsp.tile([P, D], f16, name=f"nz_{mi}", tag=f"Z{mi}")
                            nc.vector.tensor_copy(nz[:], zps[:])
                            newZ.append(nz)
                        if not last:
                            Y = newY
                        Z = newZ

                    if polish:
                        # fp32 polish: X' = 1.5 X - 0.5 X (A X^2); B = X'/sqrt(c)
                        X, Xt = [], []
                        for mi in range(MC):
                            t = ns32p.tile([P, D], f32, name=f"x32_{mi}")
                            nc.vector.tensor_copy(t[:], Z[mi][:])
                            X.append(t)
                        for mi in range(MC):
                            tps = nsps.tile([P, D], f32, name="ns_ps", tag="ns_ps")
                            for ki in range(MC):
                                nc.tensor.transpose(
                                    tps[:, _cols(ki)], X[ki][:, _cols(mi)], id32[:]
                                )
                            t = ns32p.tile([P, D], f32, name=f"xt32_{mi}")
                            nc.vector.tensor_copy(t[:], tps[:])
                            Xt.append(t)
                        Gm = []
                        for mi in range(MC):
                            gps = nsps.tile([P, D], f32, name="ns_ps", tag="ns_ps")
                            for ki in range(MC):
                                nc.tensor.matmul(
                                    gps[:], Xt[ki][:, _cols(mi)], X[ki][:],
                                    start=(ki == 0), stop=(ki == MC - 1),
                                )
                            t = ns32p.tile([P, D], f32, name=f"g32_{mi}")
                            nc.vector.tensor_copy(t[:], gps[:])
                            Gm.append(t)
                        Hm = []
                        for mi in range(MC):
                            hps = nsps.tile([P, D], f32, name="ns_ps", tag="ns_ps")
                            for ki in range(MC):
                                nc.tensor.matmul(
                                    hps[:], a32[ki][:, _cols(mi)], Gm[ki][:],
                                    start=(ki == 0), stop=(ki == MC - 1),
                                )
                            t = ns32p.tile([P, D], f32, name=f"h32_{mi}")
                            nc.vector.tensor_copy(t[:], hps[:])
                            Hm.append(t)
                        for mi in range(MC):
                            wps = nsps.tile([P, D], f32, name="ns_ps", tag="ns_ps")
                            for ki in range(MC):
                                nc.tensor.matmul(
                                    wps[:], Xt[ki][:, _cols(mi)], Hm[ki][:],
                                    start=(ki == 0), stop=(ki == MC - 1),
                                )
                            tmp = nstmpp.tile([P, D], f32, name="b_tmp", tag="b_tmp")
                            nc.vector.tensor_scalar_mul(
                                tmp[:], X[mi][:], 1.5 * INV_SQRT_C
                            )
                            ws = nstmpp.tile([P, D], f32, name="b_ws", tag="b_ws")
                            nc.vector.tensor_scalar_mul(
                                ws[:], wps[:], -0.5 * INV_SQRT_C
                            )
                            nc.vector.tensor_tensor(
                                B16[mi][:], tmp[:], ws[:], mybir.AluOpType.add
                            )
                    else:
                        for mi in range(MC):
                            nc.vector.tensor_scalar_mul(
                                B16[mi][:], Z[mi][:], INV_SQRT_C
                            )

                # ---- phase 2: whiten  y = x @ B ----
                with (
                    tc.tile_pool(name="p2y", bufs=4) as p2yp,
                    tc.tile_pool(name="p2ps", bufs=2, space="PSUM") as p2ps,
                    nc.named_scope("phase2"),
                ):
                    for c in range(nchunk):
                        xtin = p2_chunks[c]
                        ych = p2yp.tile([P, g, D], f32, name="ychunk", tag="ychunk")
                        for j0 in range(0, g, 2):
                            yps = p2ps.tile([P, 2, D], f32, name="y_ps", tag="y_ps")
                            for jj in range(2):
                                for ki in range(MC):
                                    nc.tensor.matmul(
                                        yps[:, jj, :],
                                        xtin[:, j0 + jj, _cols(ki)],
                                        B16[ki][:],
                                        start=(ki == 0), stop=(ki == MC - 1),
                                    )
                            nc.vector.tensor_copy(ych[:, j0 : j0 + 2, :], yps[:])
                        nc.sync.dma_start(y_v[:, c * g : (c + 1) * g, :], ych[:])

    nc.compile()
    return nc


def _const_inputs():
    eye = np.eye(D, dtype=np.float32)
    return {
        "eye15": (1.5 * eye).astype(np.float16),
        "eye16": eye.astype(np.float16),
        "id128_16": np.eye(P, dtype=np.float16),
        "id128_32": np.eye(P, dtype=np.float32),
    }


_PROGRAM_CACHE = {}


def kernel(x, running_covar):
    x = np.ascontiguousarray(np.asarray(x, dtype=np.float32))
    rc = np.ascontiguousarray(np.asarray(running_covar, dtype=np.float32))
    assert x.shape == (N_TOTAL, D) and rc.shape == (D, D)

    if "nc" not in _PROGRAM_CACHE:
        _PROGRAM_CACHE["nc"] = build_program()
    nc = _PROGRAM_CACHE["nc"]

    consts = _const_inputs()
    in_maps = []
    for c in range(N_CORES):
        m = {"x": x[c * SHARD : (c + 1) * SHARD], "running_covar": rc}
        m.update(consts)
        in_maps.append(m)

    res = bass_utils.run_bass_kernel_spmd(
        nc, in_maps, core_ids=list(range(N_CORES))
    )
    return np.concatenate(
        [res.results[c]["y"] for c in range(N_CORES)], axis=0
    )
